# revision 17
# baseline (speedup 1.0000x reference)
"""Causal single-head attention (B=16, T=2048, C=288, hs=32) on 8 TRN2 cores.

Reference (note the k/q swap — weights = einsum("bth,bsh->bts", k, q)):
    k = x @ Wk; q = x @ Wq; v = x @ Wv
    S[t, s] = k[t] . q[s] / sqrt(hs), causal (s <= t), softmax over s
    out = softmax(S) @ v

Sharding: data-parallel over batch, 2 batches per core, no collectives.

The ACT (scalar) engine is the roofline: exp runs only there at ~0.83ns/col
(~29us of pure columns per core) plus ~260ns/instruction.  Everything else
is organized to keep ACT densely fed:

  - Fused projection on PE with W2 = [Wk|Wq|Wv|Wq] (M=128): one pass per
    512-col group yields k@0, q@32, v@64 in kqv plus the q@96 row-group
    replica.  q@0 lands via a partition-shifted DVE copy, q@64 via one
    SBUF->SBUF DMA on the GPSIMD queue, and k@{32,64,96} via a single
    [I|I|I] replication matmul on PE + DVE cast — so the score matmuls'
    4-way PE ROW TILING (tile_position (32u,0), concurrent) has all
    operands at the right partition offsets with almost no DMA-queue
    traffic.  Input x^T rides the SP queue as fused [2,128,w]+[32,w]
    pieces (2 triggers per 512/1024-col piece).
  - PV via 2-way COLUMN TILING into a single shared [97, 512] PSUM
    accumulator bank (even chain rows 0:33 at (0,0), odd rows 64:97 at
    (0,64)).
  - Attention processes QUADS of four 128-row s-chunks in two [128, 1024]
    PSUM tiles.  Full quads: chunks (0,1)/(2,3), one 1024-wide exp per
    tile.  Diagonal quads pack (0,3)/(1,2) so each trimmed chunk owns its
    own PSUM bank -> 3 exps per diag quad, no wasted columns.
  - Group 0's quad puts all four chunks on the EVEN chain (no odd-chain
    start gap); the host uses only the even partial for t<512.
  - Projection work for group g is emitted between quads and batch 1's
    lead-in inside batch 0's tail, so the Tile scheduler can fill PE
    slack and keep ACT dense across the batch boundary.
  - Outputs ship UNNORMALIZED as [66, T] fp32 (even partial rows 0:33,
    odd rows 33:66); each chain's [33, 512] block is copied out right
    after its stop matmul and DMA'd per group; the host adds the
    partials, divides by the denominator row (ones column in V1) and
    transposes.

Softmax is computed without max-subtraction: scores are ~N(0,1) by
construction, so exp never overflows in fp32/bf16 and matches
jax.nn.softmax to rounding error.
"""

import ml_dtypes
import numpy as np

import concourse.bass as bass
import concourse.mybir as mybir
from concourse.tile import TileContext
from concourse.bass_utils import run_bass_kernel_spmd

# ---------------------------------------------------------------- constants
B, T, C, HS = 16, 2048, 288, 32
N_CORES = 8
BPC = B // N_CORES          # batches per core
P = 128                     # partition block / s-chunk size
TG = 512                    # t-columns per group (one PSUM bank of fp32)
NT = T // P                 # 16 s-chunks
NG = T // TG                # 4 t-groups
SCALE = float(HS) ** -0.5
VW = HS + 1                 # V1 chunk width (ones column appended)
VW2 = HS + 2                # padded transpose slot (4-byte PSUM alignment)
WF = 3 * HS                 # k|q|v rows in kqv
W2W = 4 * HS                # fused projection width (k|q|v|q)
TQ = 4                      # V1 transposes sharing one PSUM tile
OUTR = 2 * VW               # 66 out rows: 0:33 even partial, 33:66 odd
XPIECES = [(0, TG), (TG, TG), (2 * TG, 2 * TG)]  # x^T column pieces

COMPUTE_DT = mybir.dt.bfloat16
NP_COMPUTE_DT = np.dtype(ml_dtypes.bfloat16)


def _split_multi_waits(nc: bass.Bass) -> int:
    """This walrus build accepts only ONE sync-wait command per instruction
    (setupSyncWait<...> raises "Too many sync wait commands" otherwise), but
    Tile's semaphore assignment attaches one wait per depended-on processor.
    Move all but the last wait of each instruction onto dedicated same-engine
    NOPs placed immediately before it — the engine stalls at the NOPs first,
    so ordering semantics are identical."""
    cnt = 0
    for f in nc.m.functions:
        for bb in f.blocks:
            new_insts = []
            for inst in bb.instructions:
                si = getattr(inst, "sync_info", None)
                if si is not None and si.on_wait and len(si.on_wait) > 1:
                    extra = list(si.on_wait[:-1])
                    del si.on_wait[:-1]
                    for w in extra:
                        cnt += 1
                        new_insts.append(
                            mybir.InstNoOp(
                                name=f"{inst.name}-wsplit{cnt}",
                                sync_info=mybir.SyncInfo(on_wait=[w], on_update=[]),
                                bass_nofuse=True,
                                engine=inst.engine,
                            )
                        )
                new_insts.append(inst)
            bb.instructions[:] = new_insts
    return cnt


def _quad_layout(g: int, m: int):
    """Per-chunk (tile_idx, col_off, width) and per-tile exp regions.

    Full quads (m<g): chunks (0,1) in tile0 at [0:512]/[512:1024], (2,3) in
    tile1 likewise; one [0:1024] exp per tile.
    Diagonal quads (m==g): trimmed widths (512,384,256,128); chunks (0,3)
    in tile0 at [0:512]/[512:640], (1,2) in tile1 at [0:384]/[512:768] so
    every concurrent row-tiled matmul writes a distinct PSUM bank; exps
    [0:640] / [0:384]+[512:768] (3 instructions, no stale columns)."""
    if m < g:
        chunks = {u: (u // 2, (u % 2) * TG, TG) for u in range(4)}
        exps = [[(0, 2 * TG)], [(0, 2 * TG)]]
    else:
        chunks = {
            0: (0, 0, TG),
            3: (0, TG, TG - 3 * P),
            1: (1, 0, TG - P),
            2: (1, TG, TG - 2 * P),
        }
        exps = [[(0, TG + (TG - 3 * P))], [("strided2", TG - P)]]
    return chunks, exps


def build_attention_nc(reps: int = 1) -> bass.Bass:
    nc = bass.Bass()
    cdt = COMPUTE_DT
    f32 = mybir.dt.float32

    # host pre-packs the ragged C=288: chunks 0,1 side-by-side per
    # column piece (plain 2D DMAs; 3D patterns abort on qSP-HWDGE)
    xt_a = nc.dram_tensor("xt_a", [BPC, P, 2 * T], cdt, kind="ExternalInput")
    xt_b = nc.dram_tensor("xt_b", [BPC, C - 2 * P, T], cdt, kind="ExternalInput")
    wkqv_a = nc.dram_tensor("wkqv_a", [P, 2 * WF], cdt, kind="ExternalInput")
    wkqv_b = nc.dram_tensor("wkqv_b", [C - 2 * P, WF], cdt, kind="ExternalInput")
    ident = nc.dram_tensor("ident", [P, VW2], cdt, kind="ExternalInput")
    tri = nc.dram_tensor("tri", [P, P], cdt, kind="ExternalInput")
    ones = nc.dram_tensor("ones", [2, T], cdt, kind="ExternalInput")
    out = nc.dram_tensor("out", [BPC, OUTR, T], f32, kind="ExternalOutput")

    with TileContext(nc) as tc:
        with (
            tc.tile_pool(name="consts", bufs=1) as cpool,
            tc.tile_pool(name="xt", bufs=2) as xt_pool,
            tc.tile_pool(name="kqv", bufs=2) as kqv_pool,
            tc.tile_pool(name="v1t", bufs=2) as v1t_pool,
            tc.tile_pool(name="e", bufs=10) as e_pool,
            tc.tile_pool(name="k4", bufs=2) as k4_pool,
            tc.tile_pool(name="q4", bufs=2) as q4_pool,
            tc.tile_pool(name="outp", bufs=4) as out_pool,
            tc.tile_pool(name="ps", bufs=2, space="PSUM") as ps_pool,
            tc.tile_pool(name="pp", bufs=2, space="PSUM") as pp_pool,
            tc.tile_pool(name="po", bufs=2, space="PSUM") as po_pool,
        ):
            # weight tiles (DMAs issued inside emit_x interleaved with the
            # x^T pieces so the first projection's operands land earliest)
            w_sb = cpool.tile([P, 2 * WF], cdt, tag="w01")
            w2_sb = cpool.tile([HS, WF], cdt, tag="w2")

            def wchunk(ci):
                if ci < 2:
                    return w_sb[:, ci * WF : (ci + 1) * WF]
                return w2_sb[:]

            # remaining constants ride the GPSIMD queue, emitted inside
            # body() after batch 0's first x piece
            tri_sb = cpool.tile([P, P], cdt, tag="tri")
            ident_sb = cpool.tile([P, VW2], cdt, tag="ident")

            def emit_consts():
                nc.gpsimd.dma_start(tri_sb[:], tri[:, :])
                nc.gpsimd.dma_start(ident_sb[:], ident[:, :])

            st = {}  # per-batch emission state

            def emit_x(b):
                """x^T loads (3 column pieces, 2 fused SP triggers each),
                kqv allocation + ones rows (denominator) for batch b."""
                s = st[b] = {}
                s["xp"] = []
                for pi, (poff, psz) in enumerate(XPIECES):
                    ta = xt_pool.tile(
                        [P, 2 * psz], cdt, tag=f"xta{pi}", name=f"xta_{b}_{pi}"
                    )
                    if b == 0 and pi == 0:
                        # weights on SP; x piece 0 on the empty GPSIMD queue
                        # so the wires overlap and the first projection can
                        # start ~3us earlier
                        nc.sync.dma_start(w_sb[:], wkqv_a[:, :])
                        nc.gpsimd.dma_start(
                            ta[:, 0:psz], xt_a[b, :, 2 * poff : 2 * poff + psz]
                        )
                        nc.sync.dma_start(w2_sb[:], wkqv_b[:, :])
                        nc.gpsimd.dma_start(
                            ta[:, psz : 2 * psz],
                            xt_a[b, :, 2 * poff + psz : 2 * poff + 2 * psz],
                        )
                    else:
                        nc.sync.dma_start(
                            ta[:], xt_a[b, :, 2 * poff : 2 * poff + 2 * psz]
                        )
                    tb = xt_pool.tile(
                        [C - 2 * P, psz], cdt, tag=f"xtb{pi}", name=f"xtb_{b}_{pi}"
                    )
                    nc.sync.dma_start(tb[:], xt_b[b, :, poff : poff + psz])
                    s["xp"].append((ta, tb))
                s["kqv"] = kqv_pool.tile([WF + 2, T], cdt, tag="kqv", name=f"kqv_{b}")
                nc.gpsimd.dma_start(s["kqv"][WF : WF + 2, :], ones[:, :])
                s["k4"] = k4_pool.tile([P, T], cdt, tag="k4", name=f"k4_{b}")
                s["q4"] = q4_pool.tile([P, T], cdt, tag="q4", name=f"q4_{b}")
                s["v1t"] = v1t_pool.tile(
                    [P, NT * VW2], cdt, tag="v1t", name=f"v1t_{b}"
                )
                s["po"] = None
                s["quads"] = [(g, m) for g in range(NG) for m in range(g + 1)]
                s["qstate"] = {}

            def xpiece(b, ci, g):
                pi = min(g, 2)
                off = (g - 2) * TG if g >= 2 else 0
                ta, tb = st[b]["xp"][pi]
                if ci < 2:
                    psz = XPIECES[pi][1]
                    return ta[:, ci * psz + off : ci * psz + off + TG]
                return tb[:, off : off + TG]

            def emit_proj(b, g):
                """Fused projection group g: kqv^T [96, 512] on PE, then
                DVE row-group replicas — one fp32 cast out of PSUM plus six
                cheap SBUF->SBUF bf16 copies (4x DVE mode, partition-shifted;
                row group 0 of k and row 32 of q are read from kqv directly
                by the score matmuls)."""
                s = st[b]
                c0, c1 = g * TG, (g + 1) * TG
                pp = pp_pool.tile([WF, TG], f32, tag="pp", name=f"pp_{b}_{g}")
                for ci in range(3):
                    nc.tensor.matmul(
                        pp[:],
                        lhsT=wchunk(ci),
                        rhs=xpiece(b, ci, g),
                        start=(ci == 0),
                        stop=(ci == 2),
                    )
                q_src = s["kqv"][HS : 2 * HS, c0:c1]
                k_src = s["kqv"][0:HS, c0:c1]
                if g == 0:
                    # group 0: full cast then q@0 first (serial quad 0 needs
                    # kqv k-rows + q4 row 0 only)
                    nc.vector.tensor_copy(s["kqv"][0:WF, c0:c1], pp[0:WF, :])
                    nc.vector.tensor_copy(s["q4"][0:HS, c0:c1], q_src)
                    for u in (1, 2, 3):
                        nc.vector.tensor_copy(
                            s["k4"][u * HS : (u + 1) * HS, c0:c1], k_src
                        )
                else:
                    # k rows cast + replicated FIRST: quad (g, 0)'s scores
                    # need only these (q slices come from earlier groups)
                    nc.vector.tensor_copy(s["kqv"][0:HS, c0:c1], pp[0:HS, :])
                    for u in (1, 2, 3):
                        nc.vector.tensor_copy(
                            s["k4"][u * HS : (u + 1) * HS, c0:c1], k_src
                        )
                    nc.vector.tensor_copy(
                        s["kqv"][HS : 2 * HS, c0:c1], pp[HS : 2 * HS, :]
                    )
                    nc.vector.tensor_copy(
                        s["kqv"][2 * HS : WF, c0:c1], pp[2 * HS : WF, :]
                    )
                    nc.vector.tensor_copy(s["q4"][0:HS, c0:c1], q_src)
                nc.vector.tensor_copy(s["q4"][2 * HS : 3 * HS, c0:c1], q_src)
                nc.vector.tensor_copy(s["q4"][3 * HS : 4 * HS, c0:c1], q_src)

            def emit_transp(b, tq):
                """V1 [128, 33] for s-chunks 4tq..4tq+3 via PE transposes."""
                s = st[b]
                tp = pp_pool.tile([P, TQ * VW2], cdt, tag="pp", name=f"tp_{b}_{tq}")
                for u in range(TQ):
                    j = 4 * tq + u
                    nc.tensor.transpose(
                        tp[:, u * VW2 : (u + 1) * VW2],
                        s["kqv"][2 * HS : 2 * HS + VW2, j * P : (j + 1) * P],
                        ident_sb[2 * HS : 2 * HS + VW2, :],
                    )
                nc.vector.tensor_copy(
                    s["v1t"][:, 4 * tq * VW2 : (4 * tq + TQ) * VW2], tp[:]
                )

            def q4_slice(b, u, s0):
                if u == 1:
                    return st[b]["kqv"][HS : 2 * HS, s0 : s0 + P]
                return st[b]["q4"][32 * u : 32 * u + HS, s0 : s0 + P]

            def q0_slice(b, s0):
                return st[b]["q4"][0:HS, s0 : s0 + P]

            def k4_window(b, u, c0, c1):
                if u == 0:
                    return st[b]["kqv"][0:HS, c0:c1]
                return st[b]["k4"][32 * u : 32 * u + HS, c0:c1]

            def emit_scores(b, i):
                s = st[b]
                g, m = s["quads"][i]
                t0 = g * TG
                if m == 0:
                    s["po"] = po_pool.tile([P, TG], f32, tag="po", name=f"po_{b}_{g}")
                chunks, exps = _quad_layout(g, m)
                pss = [
                    ps_pool.tile([P, 2 * TG], f32, tag="ps", name=f"ps_{b}_{i}_{h}")
                    for h in range(2)
                ]
                es = [
                    e_pool.tile([P, 2 * TG], cdt, tag="e", name=f"e_{b}_{i}_{h}")
                    for h in range(2)
                ]
                # scores: the four K=32 matmuls run concurrently (row
                # tiling).  Batch 0's first quad instead runs serially on
                # row group 0 straight out of kqv/q4 row 0 — no replicas
                # needed, so exp starts ~2us earlier.
                serial = i == 0
                uorder = (0, 3, 1, 2) if serial else range(4)
                for u in uorder:
                    ti, off, w = chunks[u]
                    j = 4 * m + u
                    s0 = j * P
                    nc.tensor.matmul(
                        pss[ti][:, off : off + w],
                        lhsT=q0_slice(b, s0) if serial else q4_slice(b, u, s0),
                        rhs=(
                            st[b]["kqv"][0:HS, t0 + TG - w : t0 + TG]
                            if serial
                            else k4_window(b, u, t0 + TG - w, t0 + TG)
                        ),
                        start=True,
                        stop=True,
                        tile_position=(0, 0) if serial else (32 * u, 0),
                    )
                for ti in range(2):
                    for r0, r1 in exps[ti]:
                        if r0 == "strided2":
                            # two 384-col runs at stride 512: [0:384]+[512:896]
                            dst = es[ti][:, 0 : 2 * TG].rearrange(
                                "p (r w) -> p r w", w=TG
                            )[:, :, 0:r1]
                            ssrc = pss[ti][:, 0 : 2 * TG].rearrange(
                                "p (r w) -> p r w", w=TG
                            )[:, :, 0:r1]
                            nc.scalar.activation(
                                dst, ssrc, mybir.ActivationFunctionType.Exp,
                                scale=SCALE,
                            )
                            continue
                        nc.scalar.activation(
                            es[ti][:, r0:r1],
                            pss[ti][:, r0:r1],
                            mybir.ActivationFunctionType.Exp,
                            scale=SCALE,
                        )
                # causal masks on the diagonal quad's chunks (GPSIMD —
                # keeps the exp->PV path off the DVE cast bursts; the final
                # quad uses DVE, which drains ~2x faster at the tail)
                if m == g:
                    meng = nc.vector if (b == BPC - 1 and i == NG * (NG + 1) // 2 - 1) else nc.gpsimd
                    for u in range(4):
                        ti, off, w = chunks[u]
                        meng.tensor_mul(
                            es[ti][:, off : off + P],
                            es[ti][:, off : off + P],
                            tri_sb[:],
                        )
                s["qstate"][i] = (g, m, chunks, es, s["po"])

            def emit_pv(b, i):
                s = st[b]
                g, m, chunks, es, po = s["qstate"].pop(i)
                t0 = g * TG

                def ship(rows_lo, out_lo):
                    ot = out_pool.tile(
                        [VW, TG], f32, tag="ot", name=f"ot_{b}_{g}_{out_lo}"
                    )
                    nc.vector.tensor_copy(ot[:], po[rows_lo : rows_lo + VW, :])
                    nc.sync.dma_start(
                        out[b, out_lo : out_lo + VW, t0 : t0 + TG], ot[:]
                    )

                for u in range(4):
                    ti, off, w = chunks[u]
                    j = 4 * m + u
                    o = TG - w
                    rhs = es[ti][:, off : off + w]
                    lhsT = s["v1t"][:, j * VW2 : j * VW2 + VW]
                    if g == 0 or j % 2 == 0:
                        # even chain (group 0 runs entirely on it)
                        stop = j == (3 if g == 0 else 4 * g + 2)
                        nc.tensor.matmul(
                            po[0:VW, o:TG],
                            lhsT=lhsT,
                            rhs=rhs,
                            start=(j == 0),
                            stop=stop,
                            tile_position=(0, 0),
                        )
                        if stop and m == g:
                            ship(0, 0)
                    else:
                        stop = j == 4 * g + 3
                        nc.tensor.matmul(
                            po[64 : 64 + VW, o:TG],
                            lhsT=lhsT,
                            rhs=rhs,
                            start=(j == 1),
                            stop=stop,
                            tile_position=(0, 64),
                        )
                        if stop and m == g:
                            ship(64, VW)

            def body():
                # Projection for group g is emitted between quads; batch 1's
                # lead-in rides inside batch 0's last quads.  The Tile
                # scheduler dispatches by readiness with emission order as
                # priority, so this sets both deps and tie-breaks.
                def emit_quad(b, i):
                    emit_scores(b, i)
                    if i > 0:
                        emit_pv(b, i - 1)

                NQ = NG * (NG + 1) // 2

                def emit_batch_units(b):
                    # transposes trail the next stage's scores: they gate
                    # only the (lagged) PV, so scores keep exp fed first
                    emit_proj(b, 0)
                    emit_quad(b, 0)                 # serial Q(0,0)
                    emit_transp(b, 0)
                    i = 1
                    for g in range(1, NG):
                        emit_proj(b, g)
                        for k in range(g + 1 if g < NG - 1 else NG):
                            emit_quad(b, i)
                            i += 1
                            if k == 0:
                                emit_transp(b, g)
                    emit_pv(b, NQ - 1)

                emit_x(0)
                emit_consts()
                emit_x(1)
                emit_batch_units(0)
                emit_batch_units(1)

            if reps == 1:
                body()
            else:
                with tc.For_i(
                    0,
                    reps,
                    1,
                    hint_engines=(
                        mybir.EngineType.PE,
                        mybir.EngineType.Activation,
                        mybir.EngineType.DVE,
                        mybir.EngineType.SP,
                        mybir.EngineType.Pool,
                    ),
                ):
                    body()
    _split_multi_waits(nc)
    return nc


_NC_CACHE: dict = {}


def _get_nc(reps: int = 1) -> bass.Bass:
    if reps not in _NC_CACHE:
        _NC_CACHE[reps] = build_attention_nc(reps)
    return _NC_CACHE[reps]


def make_in_maps(x, Wk, Wq, Wv):
    x = np.asarray(x, dtype=np.float32)
    xt = np.ascontiguousarray(x.transpose(0, 2, 1)).astype(NP_COMPUTE_DT)
    # chunks 0,1 side-by-side per column piece; chunk 2 separate
    xt_a = np.concatenate(
        sum(
            (
                [xt[:, 0:P, poff : poff + psz], xt[:, P : 2 * P, poff : poff + psz]]
                for poff, psz in XPIECES
            ),
            [],
        ),
        axis=2,
    )
    xt_b = xt[:, 2 * P : C, :]
    wkqv = np.concatenate(
        [np.asarray(w, dtype=np.float32) for w in (Wk, Wq, Wv)], axis=1
    ).astype(NP_COMPUTE_DT)
    wkqv_a = np.concatenate([wkqv[0:P, :], wkqv[P : 2 * P, :]], axis=1)
    wkqv_b = wkqv[2 * P : C, :]
    ident = np.zeros((P, VW2), dtype=np.float32)
    ident[0:VW2, :] = np.eye(VW2)
    ident[2 * HS : 2 * HS + VW2, :] = np.eye(VW2)
    ident = ident.astype(NP_COMPUTE_DT)
    tri = np.triu(np.ones((P, P), dtype=np.float32)).astype(NP_COMPUTE_DT)
    ones = np.ones((2, T), dtype=np.float32).astype(NP_COMPUTE_DT)
    in_maps = []
    for c in range(N_CORES):
        in_maps.append(
            {
                "xt_a": np.ascontiguousarray(xt_a[c * BPC : (c + 1) * BPC]),
                "xt_b": np.ascontiguousarray(xt_b[c * BPC : (c + 1) * BPC]),
                "wkqv_a": wkqv_a,
                "wkqv_b": wkqv_b,
                "ident": ident,
                "tri": tri,
                "ones": ones,
            }
        )
    return in_maps


def _postprocess(o: np.ndarray) -> np.ndarray:
    """o: [BPC, 66, T] fp32 (unnormalized even/odd partials) ->
    [BPC, T, HS] fp32 normalized attention output.  Group 0 (t<512) runs
    entirely on the even chain; its odd rows are uninitialized."""
    num = o[:, 0:HS, :].copy()
    den = o[:, HS : HS + 1, :].copy()
    num[:, :, TG:] += o[:, VW : VW + HS, TG:]
    den[:, :, TG:] += o[:, VW + HS : VW + HS + 1, TG:]
    return np.ascontiguousarray((num / den).transpose(0, 2, 1))


def kernel(x, Wk, Wq, Wv) -> np.ndarray:
    nc = _get_nc(reps=1)
    in_maps = make_in_maps(x, Wk, Wq, Wv)
    res = run_bass_kernel_spmd(nc, in_maps, core_ids=list(range(N_CORES)))
    return np.concatenate([_postprocess(r["out"]) for r in res.results], axis=0)


# revision 27
# speedup vs baseline: 1.1019x; 1.1019x over previous
"""Causal single-head attention (B=16, T=2048, C=288, hs=32) on 8 TRN2 cores.

Reference (note the k/q swap — weights = einsum("bth,bsh->bts", k, q)):
    k = x @ Wk; q = x @ Wq; v = x @ Wv
    S[t, s] = k[t] . q[s] / sqrt(hs), causal (s <= t), softmax over s
    out = softmax(S) @ v

Sharding: data-parallel over batch, 2 batches per core, no collectives.

The ACT (scalar) engine is the roofline: exp runs only there at ~0.83ns/col
(~29us of pure columns per core) plus ~260ns/instruction.  Everything else
is organized to keep ACT densely fed:

  - Fused projection on PE with W2 = [Wk|Wq|Wv|Wq] (M=128): one pass per
    512-col group yields k@0, q@32, v@64 in kqv plus the q@96 row-group
    replica.  q@0 lands via a partition-shifted DVE copy, q@64 via one
    SBUF->SBUF DMA on the GPSIMD queue, and k@{32,64,96} via a single
    [I|I|I] replication matmul on PE + DVE cast — so the score matmuls'
    4-way PE ROW TILING (tile_position (32u,0), concurrent) has all
    operands at the right partition offsets with almost no DMA-queue
    traffic.  Input x^T rides the SP queue as fused [2,128,w]+[32,w]
    pieces (2 triggers per 512/1024-col piece).
  - PV via 2-way COLUMN TILING into a single shared [97, 512] PSUM
    accumulator bank (even chain rows 0:33 at (0,0), odd rows 64:97 at
    (0,64)).
  - Attention processes QUADS of four 128-row s-chunks in two [128, 1024]
    PSUM tiles.  Full quads: chunks (0,1)/(2,3), one 1024-wide exp per
    tile.  Diagonal quads pack (0,3)/(1,2) so each trimmed chunk owns its
    own PSUM bank -> 3 exps per diag quad, no wasted columns.
  - Group 0's quad puts all four chunks on the EVEN chain (no odd-chain
    start gap); the host uses only the even partial for t<512.
  - Projection work for group g is emitted between quads and batch 1's
    lead-in inside batch 0's tail, so the Tile scheduler can fill PE
    slack and keep ACT dense across the batch boundary.
  - Outputs ship UNNORMALIZED as [66, T] fp32 (even partial rows 0:33,
    odd rows 33:66); each chain's [33, 512] block is copied out right
    after its stop matmul and DMA'd per group; the host adds the
    partials, divides by the denominator row (ones column in V1) and
    transposes.

Softmax is computed without max-subtraction: scores are ~N(0,1) by
construction, so exp never overflows in fp32/bf16 and matches
jax.nn.softmax to rounding error.
"""

import ml_dtypes
import numpy as np

import concourse.bass as bass
import concourse.mybir as mybir
from concourse.tile import TileContext
from concourse.bass_utils import run_bass_kernel_spmd

# ---------------------------------------------------------------- constants
B, T, C, HS = 16, 2048, 288, 32
N_CORES = 8
BPC = B // N_CORES          # batches per core
P = 128                     # partition block / s-chunk size
TG = 512                    # t-columns per group (one PSUM bank of fp32)
NT = T // P                 # 16 s-chunks
NG = T // TG                # 4 t-groups
SCALE = float(HS) ** -0.5
VW = HS + 1                 # V1 chunk width (ones column appended)
VW2 = HS + 2                # padded transpose slot (4-byte PSUM alignment)
WF = 3 * HS                 # k|q|v rows in kqv
W2W = 4 * HS                # fused projection width (k|q|v|q)
TQ = 4                      # V1 transposes sharing one PSUM tile
OUTR = 2 * VW               # 66 out rows: 0:33 even partial, 33:66 odd
XPIECES = [(0, TG), (TG, TG), (2 * TG, 2 * TG)]  # x^T column pieces

COMPUTE_DT = mybir.dt.bfloat16
NP_COMPUTE_DT = np.dtype(ml_dtypes.bfloat16)


def _split_multi_waits(nc: bass.Bass) -> int:
    """This walrus build accepts only ONE sync-wait command per instruction
    (setupSyncWait<...> raises "Too many sync wait commands" otherwise), but
    Tile's semaphore assignment attaches one wait per depended-on processor.
    Move all but the last wait of each instruction onto dedicated same-engine
    NOPs placed immediately before it — the engine stalls at the NOPs first,
    so ordering semantics are identical."""
    cnt = 0
    for f in nc.m.functions:
        for bb in f.blocks:
            new_insts = []
            for inst in bb.instructions:
                si = getattr(inst, "sync_info", None)
                if si is not None and si.on_wait and len(si.on_wait) > 1:
                    extra = list(si.on_wait[:-1])
                    del si.on_wait[:-1]
                    for w in extra:
                        cnt += 1
                        new_insts.append(
                            mybir.InstNoOp(
                                name=f"{inst.name}-wsplit{cnt}",
                                sync_info=mybir.SyncInfo(on_wait=[w], on_update=[]),
                                bass_nofuse=True,
                                engine=inst.engine,
                            )
                        )
                new_insts.append(inst)
            bb.instructions[:] = new_insts
    return cnt


def _quad_layout(g: int, m: int):
    """Per-chunk (tile_idx, col_off, width) and per-tile exp regions.

    Full quads (m<g): chunks (0,1) in tile0 at [0:512]/[512:1024], (2,3) in
    tile1 likewise; one [0:1024] exp per tile.
    Diagonal quads (m==g): trimmed widths (512,384,256,128); chunks (0,3)
    in tile0 at [0:512]/[512:640], (1,2) in tile1 at [0:384]/[512:768] so
    every concurrent row-tiled matmul writes a distinct PSUM bank; exps
    [0:640] / [0:384]+[512:768] (3 instructions, no stale columns)."""
    if m < g:
        chunks = {u: (u // 2, (u % 2) * TG, TG) for u in range(4)}
        exps = [[(0, 2 * TG)], [(0, 2 * TG)]]
    else:
        chunks = {
            0: (0, 0, TG),
            3: (0, TG, TG - 3 * P),
            1: (1, 0, TG - P),
            2: (1, TG, TG - 2 * P),
        }
        exps = [[(0, TG + (TG - 3 * P))], [("strided2", TG - P)]]
    return chunks, exps


def build_attention_nc(reps: int = 1) -> bass.Bass:
    nc = bass.Bass()
    cdt = COMPUTE_DT
    f32 = mybir.dt.float32

    # host pre-packs the ragged C=288: chunks 0,1 side-by-side per
    # column piece (plain 2D DMAs; 3D patterns abort on qSP-HWDGE)
    xt_a = nc.dram_tensor("xt_a", [BPC, P, 2 * T], cdt, kind="ExternalInput")
    xt_b = nc.dram_tensor("xt_b", [BPC, C - 2 * P, T], cdt, kind="ExternalInput")
    wkqv_a = nc.dram_tensor("wkqv_a", [P, 2 * WF], cdt, kind="ExternalInput")
    wkqv_b = nc.dram_tensor("wkqv_b", [C - 2 * P, WF], cdt, kind="ExternalInput")
    ident = nc.dram_tensor("ident", [P, VW2], cdt, kind="ExternalInput")
    tri = nc.dram_tensor("tri", [P, P], cdt, kind="ExternalInput")
    ones = nc.dram_tensor("ones", [2, T], cdt, kind="ExternalInput")
    out = nc.dram_tensor("out", [BPC, OUTR, T], f32, kind="ExternalOutput")

    with TileContext(nc) as tc:
        with (
            tc.tile_pool(name="consts", bufs=1) as cpool,
            tc.tile_pool(name="xt", bufs=2) as xt_pool,
            tc.tile_pool(name="kqv", bufs=2) as kqv_pool,
            tc.tile_pool(name="v1t", bufs=2) as v1t_pool,
            tc.tile_pool(name="e", bufs=10) as e_pool,
            tc.tile_pool(name="k4", bufs=2) as k4_pool,
            tc.tile_pool(name="q4", bufs=2) as q4_pool,
            tc.tile_pool(name="outp", bufs=4) as out_pool,
            tc.tile_pool(name="ps", bufs=2, space="PSUM") as ps_pool,
            tc.tile_pool(name="pp", bufs=2, space="PSUM") as pp_pool,
            tc.tile_pool(name="po", bufs=2, space="PSUM") as po_pool,
        ):
            # weight tiles (DMAs issued inside emit_x interleaved with the
            # x^T pieces so the first projection's operands land earliest)
            w_sb = cpool.tile([P, 2 * WF], cdt, tag="w01")
            w2_sb = cpool.tile([HS, WF], cdt, tag="w2")

            def wchunk(ci):
                if ci < 2:
                    return w_sb[:, ci * WF : (ci + 1) * WF]
                return w2_sb[:]

            # remaining constants ride the GPSIMD queue, emitted inside
            # body() after batch 0's first x piece
            tri_sb = cpool.tile([P, P], cdt, tag="tri")
            ident_sb = cpool.tile([P, VW2], cdt, tag="ident")

            def emit_consts():
                nc.gpsimd.dma_start(tri_sb[:], tri[:, :])
                nc.gpsimd.dma_start(ident_sb[:], ident[:, :])

            st = {}  # per-batch emission state

            def emit_x(b):
                """x^T loads (3 column pieces, 2 fused SP triggers each),
                kqv allocation + ones rows (denominator) for batch b."""
                s = st[b] = {}
                s["xp"] = []
                for pi, (poff, psz) in enumerate(XPIECES):
                    ta = xt_pool.tile(
                        [P, 2 * psz], cdt, tag=f"xta{pi}", name=f"xta_{b}_{pi}"
                    )
                    if b == 0 and pi == 0:
                        # weights on SP; x piece 0 on the empty GPSIMD queue
                        # so the wires overlap and the first projection can
                        # start ~3us earlier
                        nc.sync.dma_start(w_sb[:], wkqv_a[:, :])
                        nc.gpsimd.dma_start(
                            ta[:, 0:psz], xt_a[b, :, 2 * poff : 2 * poff + psz]
                        )
                        nc.sync.dma_start(w2_sb[:], wkqv_b[:, :])
                        nc.gpsimd.dma_start(
                            ta[:, psz : 2 * psz],
                            xt_a[b, :, 2 * poff + psz : 2 * poff + 2 * psz],
                        )
                    else:
                        nc.sync.dma_start(
                            ta[:], xt_a[b, :, 2 * poff : 2 * poff + 2 * psz]
                        )
                    tb = xt_pool.tile(
                        [C - 2 * P, psz], cdt, tag=f"xtb{pi}", name=f"xtb_{b}_{pi}"
                    )
                    nc.sync.dma_start(tb[:], xt_b[b, :, poff : poff + psz])
                    s["xp"].append((ta, tb))
                s["kqv"] = kqv_pool.tile([WF + 2, T], cdt, tag="kqv", name=f"kqv_{b}")
                nc.gpsimd.dma_start(s["kqv"][WF : WF + 2, :], ones[:, :])
                s["k4"] = k4_pool.tile([P, T], cdt, tag="k4", name=f"k4_{b}")
                s["q4"] = q4_pool.tile([P, T], cdt, tag="q4", name=f"q4_{b}")
                s["v1t"] = v1t_pool.tile(
                    [P, NT * VW2], cdt, tag="v1t", name=f"v1t_{b}"
                )
                s["po"] = None
                s["quads"] = [(g, m) for g in range(NG) for m in range(g + 1)]
                s["qstate"] = {}

            def xpiece(b, ci, g):
                pi = min(g, 2)
                off = (g - 2) * TG if g >= 2 else 0
                ta, tb = st[b]["xp"][pi]
                if ci < 2:
                    psz = XPIECES[pi][1]
                    return ta[:, ci * psz + off : ci * psz + off + TG]
                return tb[:, off : off + TG]

            def emit_proj(b, g):
                """Fused projection group g: kqv^T [96, 512] on PE, then
                DVE row-group replicas — one fp32 cast out of PSUM plus six
                cheap SBUF->SBUF bf16 copies (4x DVE mode, partition-shifted;
                row group 0 of k and row 32 of q are read from kqv directly
                by the score matmuls)."""
                s = st[b]
                c0, c1 = g * TG, (g + 1) * TG
                pp = pp_pool.tile([WF, TG], f32, tag="pp", name=f"pp_{b}_{g}")
                for ci in range(3):
                    nc.tensor.matmul(
                        pp[:],
                        lhsT=wchunk(ci),
                        rhs=xpiece(b, ci, g),
                        start=(ci == 0),
                        stop=(ci == 2),
                    )
                nc.vector.tensor_copy(s["kqv"][0:WF, c0:c1], pp[0:WF, :])
                q_src = s["kqv"][HS : 2 * HS, c0:c1]
                k_src = s["kqv"][0:HS, c0:c1]
                nc.vector.tensor_copy(s["q4"][0:HS, c0:c1], q_src)
                nc.vector.tensor_copy(s["k4"][HS : 2 * HS, c0:c1], k_src)
                nc.vector.tensor_copy(s["k4"][2 * HS : 3 * HS, c0:c1], k_src)
                nc.vector.tensor_copy(s["k4"][3 * HS : 4 * HS, c0:c1], k_src)
                nc.vector.tensor_copy(s["q4"][2 * HS : 3 * HS, c0:c1], q_src)
                nc.vector.tensor_copy(s["q4"][3 * HS : 4 * HS, c0:c1], q_src)

            def emit_transp(b, tq):
                """V1 [128, 33] for s-chunks 4tq..4tq+3 via PE transposes."""
                s = st[b]
                tp = pp_pool.tile([P, TQ * VW2], cdt, tag="pp", name=f"tp_{b}_{tq}")
                for u in range(TQ):
                    j = 4 * tq + u
                    nc.tensor.transpose(
                        tp[:, u * VW2 : (u + 1) * VW2],
                        s["kqv"][2 * HS : 2 * HS + VW2, j * P : (j + 1) * P],
                        ident_sb[2 * HS : 2 * HS + VW2, :],
                    )
                nc.vector.tensor_copy(
                    s["v1t"][:, 4 * tq * VW2 : (4 * tq + TQ) * VW2], tp[:]
                )

            def q4_slice(b, u, s0):
                if u == 1:
                    return st[b]["kqv"][HS : 2 * HS, s0 : s0 + P]
                return st[b]["q4"][32 * u : 32 * u + HS, s0 : s0 + P]

            def q0_slice(b, s0):
                return st[b]["q4"][0:HS, s0 : s0 + P]

            def k4_window(b, u, c0, c1):
                if u == 0:
                    return st[b]["kqv"][0:HS, c0:c1]
                return st[b]["k4"][32 * u : 32 * u + HS, c0:c1]

            def emit_scores(b, i):
                s = st[b]
                g, m = s["quads"][i]
                t0 = g * TG
                if m == 0:
                    s["po"] = po_pool.tile([P, TG], f32, tag="po", name=f"po_{b}_{g}")
                chunks, exps = _quad_layout(g, m)
                if b == BPC - 1 and i == NG * (NG + 1) // 2 - 1:
                    exps = [
                        [(0, TG), (TG, TG + (TG - 3 * P))],
                        [(0, TG - P), (TG, TG + (TG - 2 * P))],
                    ]
                pss = [
                    ps_pool.tile([P, 2 * TG], f32, tag="ps", name=f"ps_{b}_{i}_{h}")
                    for h in range(2)
                ]
                es = [
                    e_pool.tile([P, 2 * TG], cdt, tag="e", name=f"e_{b}_{i}_{h}")
                    for h in range(2)
                ]
                # scores: the four K=32 matmuls run concurrently (row
                # tiling).  Batch 0's first quad instead runs serially on
                # row group 0 straight out of kqv/q4 row 0 — no replicas
                # needed, so exp starts ~2us earlier.
                serial = b == 0 and i == 0
                uorder = (0, 3, 1, 2) if serial else range(4)
                for u in uorder:
                    ti, off, w = chunks[u]
                    j = 4 * m + u
                    s0 = j * P
                    nc.tensor.matmul(
                        pss[ti][:, off : off + w],
                        lhsT=q0_slice(b, s0) if serial else q4_slice(b, u, s0),
                        rhs=(
                            st[b]["kqv"][0:HS, t0 + TG - w : t0 + TG]
                            if serial
                            else k4_window(b, u, t0 + TG - w, t0 + TG)
                        ),
                        start=True,
                        stop=True,
                        tile_position=(0, 0) if serial else (32 * u, 0),
                    )
                for ti in range(2):
                    for r0, r1 in exps[ti]:
                        if r0 == "strided2":
                            # two 384-col runs at stride 512: [0:384]+[512:896]
                            dst = es[ti][:, 0 : 2 * TG].rearrange(
                                "p (r w) -> p r w", w=TG
                            )[:, :, 0:r1]
                            ssrc = pss[ti][:, 0 : 2 * TG].rearrange(
                                "p (r w) -> p r w", w=TG
                            )[:, :, 0:r1]
                            nc.scalar.activation(
                                dst, ssrc, mybir.ActivationFunctionType.Exp,
                                scale=SCALE,
                            )
                            continue
                        nc.scalar.activation(
                            es[ti][:, r0:r1],
                            pss[ti][:, r0:r1],
                            mybir.ActivationFunctionType.Exp,
                            scale=SCALE,
                        )
                # causal masks on the diagonal quad's chunks (GPSIMD —
                # keeps the exp->PV path off the DVE cast bursts; the final
                # quad uses DVE, which drains ~2x faster at the tail)
                if m == g:
                    meng = nc.vector if (b == BPC - 1 and i == NG * (NG + 1) // 2 - 1) else nc.gpsimd
                    for u in range(4):
                        ti, off, w = chunks[u]
                        meng.tensor_mul(
                            es[ti][:, off : off + P],
                            es[ti][:, off : off + P],
                            tri_sb[:],
                        )
                s["qstate"][i] = (g, m, chunks, es, s["po"])

            def emit_pv(b, i):
                s = st[b]
                g, m, chunks, es, po = s["qstate"].pop(i)
                t0 = g * TG

                def ship(rows_lo, out_lo):
                    ot = out_pool.tile(
                        [VW, TG], f32, tag="ot", name=f"ot_{b}_{g}_{out_lo}"
                    )
                    nc.vector.tensor_copy(ot[:], po[rows_lo : rows_lo + VW, :])
                    nc.sync.dma_start(
                        out[b, out_lo : out_lo + VW, t0 : t0 + TG], ot[:]
                    )

                for u in ((0, 1, 3, 2) if m == g and g > 0 else range(4)):
                    ti, off, w = chunks[u]
                    j = 4 * m + u
                    o = TG - w
                    rhs = es[ti][:, off : off + w]
                    lhsT = s["v1t"][:, j * VW2 : j * VW2 + VW]
                    if g == 0 or j % 2 == 0:
                        # even chain (group 0 runs entirely on it)
                        stop = j == (3 if g == 0 else 4 * g + 2)
                        nc.tensor.matmul(
                            po[0:VW, o:TG],
                            lhsT=lhsT,
                            rhs=rhs,
                            start=(j == 0),
                            stop=stop,
                            tile_position=(0, 0),
                        )
                        if stop and m == g:
                            ship(0, 0)
                    else:
                        stop = j == 4 * g + 3
                        nc.tensor.matmul(
                            po[64 : 64 + VW, o:TG],
                            lhsT=lhsT,
                            rhs=rhs,
                            start=(j == 1),
                            stop=stop,
                            tile_position=(0, 64),
                        )
                        if stop and m == g:
                            ship(64, VW)

            def body():
                # Projection for group g is emitted between quads; batch 1's
                # lead-in rides inside batch 0's last quads.  The Tile
                # scheduler dispatches by readiness with emission order as
                # priority, so this sets both deps and tie-breaks.
                def emit_quad(b, i):
                    emit_scores(b, i)
                    if i > 0:
                        emit_pv(b, i - 1)

                NQ = NG * (NG + 1) // 2

                def emit_batch_units(b):
                    # projections outrank transposes: P_g gates stage-g
                    # scores (the exp feed), T_g only gates the lagged PV
                    emit_proj(b, 0)
                    emit_quad(b, 0)                 # serial Q(0,0)
                    emit_proj(b, 1)
                    emit_transp(b, 0)
                    emit_quad(b, 1)                 # Q(1,0)
                    emit_proj(b, 2)
                    emit_quad(b, 2)                 # Q(1,1)
                    emit_transp(b, 1)               # <- before Q3 (pv of 1,1)
                    emit_quad(b, 3)                 # Q(2,0)
                    emit_proj(b, 3)
                    emit_quad(b, 4)                 # Q(2,1)
                    emit_quad(b, 5)                 # Q(2,2)
                    emit_transp(b, 2)               # <- before Q6 (pv of 2,2)
                    emit_quad(b, 6)                 # Q(3,0)
                    emit_quad(b, 7)                 # Q(3,1)
                    emit_quad(b, 8)                 # Q(3,2)
                    emit_transp(b, 3)               # <- before Q9 (final pv)
                    emit_quad(b, 9)                 # Q(3,3)
                    emit_pv(b, NQ - 1)

                emit_x(0)
                emit_consts()
                emit_x(1)
                emit_batch_units(0)
                emit_batch_units(1)

            if reps == 1:
                body()
            else:
                with tc.For_i(
                    0,
                    reps,
                    1,
                    hint_engines=(
                        mybir.EngineType.PE,
                        mybir.EngineType.Activation,
                        mybir.EngineType.DVE,
                        mybir.EngineType.SP,
                        mybir.EngineType.Pool,
                    ),
                ):
                    body()
    _split_multi_waits(nc)
    return nc


_NC_CACHE: dict = {}


def _get_nc(reps: int = 1) -> bass.Bass:
    if reps not in _NC_CACHE:
        _NC_CACHE[reps] = build_attention_nc(reps)
    return _NC_CACHE[reps]


def make_in_maps(x, Wk, Wq, Wv):
    x = np.asarray(x, dtype=np.float32)
    xt = np.ascontiguousarray(x.transpose(0, 2, 1)).astype(NP_COMPUTE_DT)
    # chunks 0,1 side-by-side per column piece; chunk 2 separate
    xt_a = np.concatenate(
        sum(
            (
                [xt[:, 0:P, poff : poff + psz], xt[:, P : 2 * P, poff : poff + psz]]
                for poff, psz in XPIECES
            ),
            [],
        ),
        axis=2,
    )
    xt_b = xt[:, 2 * P : C, :]
    wkqv = np.concatenate(
        [np.asarray(w, dtype=np.float32) for w in (Wk, Wq, Wv)], axis=1
    ).astype(NP_COMPUTE_DT)
    wkqv_a = np.concatenate([wkqv[0:P, :], wkqv[P : 2 * P, :]], axis=1)
    wkqv_b = wkqv[2 * P : C, :]
    ident = np.zeros((P, VW2), dtype=np.float32)
    ident[0:VW2, :] = np.eye(VW2)
    ident[2 * HS : 2 * HS + VW2, :] = np.eye(VW2)
    ident = ident.astype(NP_COMPUTE_DT)
    tri = np.triu(np.ones((P, P), dtype=np.float32)).astype(NP_COMPUTE_DT)
    ones = np.ones((2, T), dtype=np.float32).astype(NP_COMPUTE_DT)
    in_maps = []
    for c in range(N_CORES):
        in_maps.append(
            {
                "xt_a": np.ascontiguousarray(xt_a[c * BPC : (c + 1) * BPC]),
                "xt_b": np.ascontiguousarray(xt_b[c * BPC : (c + 1) * BPC]),
                "wkqv_a": wkqv_a,
                "wkqv_b": wkqv_b,
                "ident": ident,
                "tri": tri,
                "ones": ones,
            }
        )
    return in_maps


def _postprocess(o: np.ndarray) -> np.ndarray:
    """o: [BPC, 66, T] fp32 (unnormalized even/odd partials) ->
    [BPC, T, HS] fp32 normalized attention output.  Group 0 (t<512) runs
    entirely on the even chain; its odd rows are uninitialized."""
    num = o[:, 0:HS, :].copy()
    den = o[:, HS : HS + 1, :].copy()
    num[:, :, TG:] += o[:, VW : VW + HS, TG:]
    den[:, :, TG:] += o[:, VW + HS : VW + HS + 1, TG:]
    return np.ascontiguousarray((num / den).transpose(0, 2, 1))


def kernel(x, Wk, Wq, Wv) -> np.ndarray:
    nc = _get_nc(reps=1)
    in_maps = make_in_maps(x, Wk, Wq, Wv)
    res = run_bass_kernel_spmd(nc, in_maps, core_ids=list(range(N_CORES)))
    return np.concatenate([_postprocess(r["out"]) for r in res.results], axis=0)
